# revision 28
# baseline (speedup 1.0000x reference)
"""DeepseekV2 MLA attention (T=2048, H=16) on 8 trn2 cores.

v2: stage-1 low-rank a-projections are SHARDED over tokens (each core
computes q_c/kv_c/k_pe for its 256-token slice), normalized + roped
locally, then exchanged with a single bf16 AllGather into a Shared DRAM
buffer. Downstream (per-head up-projections, attention, o_proj) is
tensor-parallel over heads (2 heads/core) exactly as v1; per-core
partial o_proj outputs are summed on the host.

Device-side layout tricks (kept from v1):
- All attention operands in "transposed" [feature, t] layout; matmul
  contractions land on the partition dim with no PE transposes.
- Scores as S^T[k, q] = K^T q; softmax denominator via ones-matmul;
  no row-max subtraction; normalization applied after P@V.
- RMSNorm r[t] computed via squares + ones-matmul; applied to the
  stage-1 outputs BEFORE the gather (ln weights folded into b-projs).
- Neox rope folded into duplicated/rotated weight columns.
- A tiny warmup AllGather issues first so CC setup overlaps stage-1.
"""

import numpy as np

T = 2048
HID = 2048
H = 16
NC_ = 8
HLOC = H // NC_          # 2 heads per core
TLOC = T // NC_          # 256 tokens per core
QL = 1536                # q lora
KVL = 512                # kv lora
DN = 128                 # nope dim
DR = 64                  # rope dim
DQK = DN + DR            # 192
DV = 128
EPS = 1e-6
SCALE = float(DQK) ** -0.5
P = 128
NM1 = 18                 # stage-1 m-tiles: 12 q + 4 kv + ropeA + ropeB
NMB = 17                 # bundle m-tiles: 12 q + 4 kv + roped kpe dup
CH = 512                 # stage-2 t-chunk
NCH = T // CH
QC = 512                 # attention q-chunk
NQC = T // QC
NKB = T // P             # key blocks
NKQ = QL // P            # 12
NKV = KVL // P           # 4

_CACHE = {}
LAST_RESULTS = None


def _split_multi_waits(nc, mybir):
    """Walrus embeds at most one sem/event wait per TPB instruction; hoist
    extra waits onto preceding same-engine NoOps (queue FIFO keeps order)."""
    n = 0
    for f in nc.m.functions:
        for bb in f.blocks:
            new = []
            for inst in bb.instructions:
                si = getattr(inst, "sync_info", None)
                if si is not None and len(si.on_wait) > 1:
                    waits = list(si.on_wait)
                    for i, wv in enumerate(waits[:-1]):
                        noop = mybir.InstNoOp(
                            name=f"{inst.name}-wsplit{i}",
                            engine=inst.engine,
                            ins=[],
                            outs=[],
                        )
                        noop.bass_nofuse = True
                        noop.sync_info = mybir.SyncInfo(on_wait=[wv], on_update=[])
                        new.append(noop)
                    inst.sync_info = mybir.SyncInfo(
                        on_wait=[waits[-1]], on_update=list(si.on_update)
                    )
                    n += 1
                new.append(inst)
            bb.instructions = new
    return n


def _build_program():
    import concourse.bass as bass
    import concourse.tile as tile
    from concourse import mybir

    f32 = mybir.dt.float32
    bf16 = mybir.dt.bfloat16
    f32r = mybir.dt.float32r
    AF = mybir.ActivationFunctionType

    nc = bass.Bass(num_devices=NC_)

    # per-core token slice of hidden^T, pre-tiled [p, k, t]
    hL_d = nc.declare_dram_parameter("hL", [P, HID // P, TLOC], bf16, isOutput=False)
    # stacked stage-1 weights: 12 q | 4 kv latent | ropeA dup | ropeB dup
    wS_d = nc.declare_dram_parameter("wS", [P, NM1, HID // P, P], bf16, isOutput=False)
    # per-core rope tables for MY tokens (dup'd to 128 rows)
    cosL_d = nc.declare_dram_parameter("cosL", [P, TLOC], f32, isOutput=False)
    sinL_d = nc.declare_dram_parameter("sinL", [P, TLOC], f32, isOutput=False)
    # h0_nope 128 | h1_nope 128 | ropeA 128 | ropeB 128  (ln folded)
    wqb_d = nc.declare_dram_parameter("wqb", [P, 4, NKQ, P], bf16, isOutput=False)
    wkvbk_d = nc.declare_dram_parameter("wkvbk", [P, NKV, HLOC * DN], bf16, isOutput=False)
    wkvbv_d = nc.declare_dram_parameter("wkvbv", [P, NKV, HLOC * DV], bf16, isOutput=False)
    wo_d = nc.declare_dram_parameter("wo", [P, HLOC, HID], f32r, isOutput=False)
    cos2_d = nc.declare_dram_parameter("cos2", [P, T], f32, isOutput=False)
    sin2_d = nc.declare_dram_parameter("sin2", [P, T], f32, isOutput=False)
    trimask_d = nc.declare_dram_parameter("trimask", [P, P], bf16, isOutput=False)
    y_d = nc.declare_dram_parameter("y", [T, HID], f32, isOutput=True)

    out_kv = nc.dram_tensor("gather_kv", [NC_, P, 5, TLOC], bf16, addr_space="Shared")
    out_q = nc.dram_tensor("gather_q", [NC_, P, NKQ, TLOC], bf16, addr_space="Shared")
    warm_sh = nc.dram_tensor("warm_out", [NC_, 1, 16], bf16, addr_space="Shared")

    def r32(ap):
        return ap.bitcast(f32r)

    with tile.TileContext(nc) as tc, nc.allow_low_precision(
        reason="bf16 latent exchange + fp32r PE operands are intentional"
    ):
        with (
            tc.tile_pool(name="persist", bufs=1) as pp,
            tc.tile_pool(name="dramp", bufs=1, space="DRAM") as dramp,
        ):
            # ---- warmup collective: absorbs CC channel setup / core skew ----
            warm_in = dramp.tile([1, 16], bf16, name="warmin")
            warm_sb = pp.tile([1, 16], bf16, name="warmsb")
            nc.vector.memset(warm_sb, 0.0)
            nc.sync.dma_start(out=warm_in, in_=warm_sb)
            nc.gpsimd.collective_compute(
                "AllGather",
                mybir.AluOpType.bypass,
                replica_groups=[list(range(NC_))],
                ins=[warm_in[:, :].opt()],
                outs=[warm_sh[:, :, :].opt()],
            )

            # persistent SBUF tensors (DMAs issued on the scalar queue, off the
            # critical stage-1 sync queue; wo/trimask emission deferred)
            wkvbk_sb = pp.tile([P, NKV, HLOC * DN], bf16, name="wkvbk")
            nc.scalar.dma_start(out=wkvbk_sb, in_=wkvbk_d[:, :, :])
            wkvbv_sb = pp.tile([P, NKV, HLOC * DV], bf16, name="wkvbv")
            nc.scalar.dma_start(out=wkvbv_sb, in_=wkvbv_d[:, :, :])
            wqb_sb = pp.tile([P, 4, NKQ, P], bf16, name="wqb")
            wo_sb = pp.tile([P, HLOC, T], f32r, name="wo")
            trimask_sb = pp.tile([P, P], bf16, name="trimask")
            ones_f = pp.tile([P, P], f32, name="ones_f")
            nc.vector.memset(ones_f, 1.0)
            ones_sb = pp.tile([P, 1], f32r, name="ones")
            nc.vector.tensor_copy(ones_sb, ones_f[:, 0:1])
            ones_b = pp.tile([P, 1], bf16, name="ones_b")
            nc.vector.tensor_copy(ones_b, ones_f[:, 0:1])
            col_ones = pp.tile([1, P], f32r, name="col_ones")
            nc.vector.tensor_copy(col_ones, ones_f[0:1, :])
            zmask = pp.tile([P, HLOC], f32, name="zmask")
            nc.vector.memset(zmask[0:DR, 0:1], 1.0)
            nc.vector.memset(zmask[DR:P, 0:1], 0.0)
            nc.vector.memset(zmask[0:DR, 1:2], 0.0)
            nc.vector.memset(zmask[DR:P, 1:2], 1.0)
            eps_sb = pp.tile([1, 1], f32, name="eps")
            nc.vector.memset(eps_sb, EPS)

            qTn = [pp.tile([P, T], f32r, name=f"qTn{h}") for h in range(HLOC)]
            qpeT2 = pp.tile([P, T], f32r, name="qpeT2")
            KT = [pp.tile([P, T], f32r, name=f"KT{h}") for h in range(HLOC)]
            kpe2 = [pp.tile([P, T], f32r, name=f"kpe2{h}") for h in range(HLOC)]
            V_sb = [pp.tile([P, HLOC * DV], bf16, name=f"v{i}") for i in range(NKB)]

            # -------- Stage 1 + 2: sharded projections, pipelined gathers ------
            with (
                tc.tile_pool(name="s1loc", bufs=1) as lp,
                tc.tile_pool(name="s1stream", bufs=3) as sp_,
                tc.tile_pool(name="s1small", bufs=1) as smp,
                tc.tile_pool(name="kvread", bufs=4) as kvr,
                tc.tile_pool(name="s2chunk", bufs=2) as cp_,
                tc.tile_pool(name="s2stream", bufs=2) as sp2,
                tc.tile_pool(name="s2small", bufs=2) as sm2,
                tc.tile_pool(name="s1ps", bufs=2, space="PSUM") as s1ps,
                tc.tile_pool(name="ssqps", bufs=1, space="PSUM") as ssqps,
                tc.tile_pool(name="upps", bufs=4, space="PSUM") as upps,
            ):
                h_sb = lp.tile([P, HID // P, TLOC], bf16, name="hloc")
                nc.sync.dma_start(out=h_sb, in_=hL_d[:, :, :])
                cosL_sb = smp.tile([P, TLOC], f32, name="cosL")
                nc.sync.dma_start(out=cosL_sb, in_=cosL_d[:, :])
                sinL_sb = smp.tile([P, TLOC], f32, name="sinL")
                nc.sync.dma_start(out=sinL_sb, in_=sinL_d[:, :])

                # m-order: kv latent 0..3 | ropeA 4 | ropeB 5 | q 6..17
                stage = lp.tile([P, NM1, TLOC], bf16, name="stage")
                bkv = lp.tile([P, 5, TLOC], bf16, name="bkv")
                bq = lp.tile([P, NKQ, TLOC], bf16, name="bq")
                inkv_b = dramp.tile([P, 5, TLOC], bf16, name="bounce_kv")
                inq_b = dramp.tile([P, NKQ, TLOC], bf16, name="bounce_q")

                sqacc_kv = smp.tile([P, TLOC], f32r, name="sqkv")
                sqacc_q = smp.tile([P, TLOC], f32r, name="sqq")
                sqt = smp.tile([P, TLOC], f32r, name="sqt", bufs=2)

                def rms_scale(sqacc, inv_n, name):
                    """rsqrt(mean+eps) broadcast to 128 rows, via one
                    ones-matmul (partition sum) + wide DVE ops."""
                    ssq = ssqps.tile([1, TLOC], f32, name=f"ssq{name}")
                    nc.tensor.matmul(
                        ssq, lhsT=r32(ones_sb), rhs=sqacc, start=True, stop=True
                    )
                    rr = smp.tile([1, TLOC], f32r, name=f"r{name}")
                    nc.scalar.activation(
                        rr, ssq, func=AF.Sqrt, bias=eps_sb, scale=inv_n
                    )
                    rb_ps = s1ps.tile([P, TLOC], f32, name="s1")
                    nc.tensor.matmul(rb_ps, lhsT=col_ones, rhs=rr, start=True, stop=True)
                    rb = smp.tile([P, TLOC], f32, name=f"rb{name}")
                    nc.vector.reciprocal(rb, rb_ps)
                    return rb

                def s1_tile(m, sqacc, first_sq):
                    w_sb = sp_.tile([P, HID // P, P], bf16, name="wstream")
                    nc.sync.dma_start(out=w_sb, in_=wS_d[:, m, :, :])
                    ps = s1ps.tile([P, TLOC], f32, name="s1")
                    for k in range(HID // P):
                        nc.tensor.matmul(
                            ps,
                            lhsT=w_sb[:, k, :],
                            rhs=h_sb[:, k, :],
                            start=(k == 0),
                            stop=(k == HID // P - 1),
                        )
                    nc.scalar.copy(stage[:, m, :], ps)
                    if sqacc is not None:
                        if first_sq:
                            nc.scalar.square(sqacc, ps)
                        else:
                            nc.scalar.square(sqt, ps)
                            nc.vector.tensor_add(sqacc, sqacc, sqt)

                # ---- part A: kv latent + rope tiles, kv gather ----
                for m in range(4):
                    s1_tile(m, sqacc_kv, m == 0)
                for m in range(4, 6):
                    s1_tile(m, None, False)
                rkv_b = rms_scale(sqacc_kv, 1.0 / KVL, "kv")
                for m in range(NKV):
                    nc.vector.tensor_mul(bkv[:, m, :], stage[:, m, :], rkv_b)
                t1 = smp.tile([P, TLOC], f32, name="ropet1")
                t2 = smp.tile([P, TLOC], f32, name="ropet2")
                nc.vector.tensor_mul(t1, stage[:, 4, :], cosL_sb)
                nc.vector.tensor_mul(t2, stage[:, 5, :], sinL_sb)
                nc.vector.tensor_add(bkv[:, 4, :], t1, t2)
                nc.sync.dma_start(out=inkv_b, in_=bkv)
                nc.gpsimd.collective_compute(
                    "AllGather",
                    mybir.AluOpType.bypass,
                    replica_groups=[list(range(NC_))],
                    ins=[inkv_b[:, :, :].opt()],
                    outs=[out_kv[:, :, :, :].opt()],
                )
                # deferred persistent weight loads (scalar queue, during gathers)
                nc.scalar.dma_start(out=wqb_sb, in_=wqb_d[:, :, :, :])
                nc.scalar.dma_start(out=wo_sb, in_=wo_d[:, :, :])
                nc.scalar.dma_start(out=trimask_sb, in_=trimask_d[:, :])

                # kv chunk read-back (gpsimd: queued right after the kv gather)
                kvc = []
                for c in range(NCH):
                    kt = kvr.tile([P, 5, CH], bf16, name="kvchunk")
                    for s in range(CH // TLOC):
                        nc.gpsimd.dma_start(
                            out=kt[:, :, s * TLOC : (s + 1) * TLOC],
                            in_=out_kv[c * (CH // TLOC) + s, :, :, :],
                        )
                    kvc.append(kt)

                # ---- part B: q tiles, q gather ----
                for m in range(6, NM1):
                    s1_tile(m, sqacc_q, m == 6)
                rq_b = rms_scale(sqacc_q, 1.0 / QL, "q")
                for m in range(NKQ):
                    nc.vector.tensor_mul(bq[:, m, :], stage[:, 6 + m, :], rq_b)
                nc.sync.dma_start(out=inq_b, in_=bq)
                nc.gpsimd.collective_compute(
                    "AllGather",
                    mybir.AluOpType.bypass,
                    replica_groups=[list(range(NC_))],
                    ins=[inq_b[:, :, :].opt()],
                    outs=[out_q[:, :, :, :].opt()],
                )

                # ---- K/V/kpe from gathered kv latents (overlaps q gather) ----
                for c in range(NCH):
                    t0 = c * CH
                    kvp_sb = kvc[c]
                    for h in range(HLOC):
                        nc.vector.tensor_scalar_mul(
                            kpe2[h][:, t0 : t0 + CH],
                            kvp_sb[:, 4, :],
                            zmask[:, h : h + 1],
                        )
                    for h in range(HLOC):
                        ps = upps.tile([P, CH], f32, name="up")
                        for k in range(NKV):
                            nc.tensor.matmul(
                                ps,
                                lhsT=wkvbk_sb[:, k, h * P : (h + 1) * P],
                                rhs=kvp_sb[:, k, :],
                                start=(k == 0),
                                stop=(k == NKV - 1),
                            )
                        nc.vector.tensor_copy(KT[h][:, t0 : t0 + CH], ps)
                    for tt in range(CH // P):
                        ps = upps.tile([P, HLOC * DV], f32, name="up")
                        for k in range(NKV):
                            nc.tensor.matmul(
                                ps,
                                lhsT=kvp_sb[:, k, tt * P : (tt + 1) * P],
                                rhs=wkvbv_sb[:, k, :],
                                start=(k == 0),
                                stop=(k == NKV - 1),
                            )
                        nc.vector.tensor_copy(V_sb[(t0 // P) + tt], ps)

                # ---- q up-projection from gathered q_c ----
                for c in range(NCH):
                    t0 = c * CH
                    qc_sb = cp_.tile([P, NKQ, CH], bf16, name="qcchunk")
                    for s in range(CH // TLOC):
                        nc.gpsimd.dma_start(
                            out=qc_sb[:, :, s * TLOC : (s + 1) * TLOC],
                            in_=out_q[c * (CH // TLOC) + s, :, :, :],
                        )
                    cos_sb = sm2.tile([P, CH], f32, name="cosc")
                    nc.sync.dma_start(out=cos_sb, in_=cos2_d[:, t0 : t0 + CH])
                    sin_sb = sm2.tile([P, CH], f32, name="sinc")
                    nc.sync.dma_start(out=sin_sb, in_=sin2_d[:, t0 : t0 + CH])

                    ups = []
                    for mo in range(4):
                        ps = upps.tile([P, CH], f32, name="up")
                        for k in range(NKQ):
                            nc.tensor.matmul(
                                ps,
                                lhsT=wqb_sb[:, mo, k, :],
                                rhs=qc_sb[:, k, :],
                                start=(k == 0),
                                stop=(k == NKQ - 1),
                            )
                        ups.append(ps)
                        if mo < HLOC:
                            nc.vector.tensor_copy(qTn[mo][:, t0 : t0 + CH], ps)
                    t3 = sm2.tile([P, CH], f32, name="ropet3")
                    t4 = sm2.tile([P, CH], f32, name="ropet4")
                    nc.vector.tensor_mul(t3, ups[2], cos_sb)
                    nc.vector.tensor_mul(t4, ups[3], sin_sb)
                    nc.vector.tensor_add(qpeT2[:, t0 : t0 + CH], t3, t4)

            # ---------------- Stage 3: attention ----------------
            with (
                tc.tile_pool(name="bpt", bufs=4) as ptp,
                tc.tile_pool(name="bsmall", bufs=3) as bsm,
                tc.tile_pool(name="sps", bufs=2, space="PSUM") as spsp,
                tc.tile_pool(name="otps", bufs=2, space="PSUM") as otpsp,
                tc.tile_pool(name="lps", bufs=2, space="PSUM") as lpsp,
            ):
                OT_sb = [
                    [ptp.tile([P, QC], f32r, name=f"ot{h}_{j}", bufs=1) for j in range(NQC)]
                    for h in range(HLOC)
                ]

                def emit_oproj(j):
                    # all heads' OT for q-block j are normalized; project now
                    # (fills PE bubbles between score/exp pairs)
                    for tt in range(4 * j, 4 * j + 4):
                        sub = (tt % 4) * P
                        for n in range(HID // QC):
                            y_ps = spsp.tile([P, 2 * QC], f32, name="sps2")[:, :QC]
                            for h in range(HLOC):
                                nc.tensor.matmul(
                                    y_ps,
                                    lhsT=r32(OT_sb[h][j][:, sub : sub + P]),
                                    rhs=r32(wo_sb[:, h, n * QC : (n + 1) * QC]),
                                    start=(h == 0),
                                    stop=(h == HLOC - 1),
                                )
                            y_sb = ptp.tile([P, QC], f32, name="ysb")
                            if (tt * (HID // QC) + n) % 2 == 0:
                                nc.vector.tensor_copy(y_sb, y_ps)
                            else:
                                nc.scalar.copy(y_sb, y_ps)
                            nc.sync.dma_start(
                                out=y_d[tt * P : (tt + 1) * P, n * QC : (n + 1) * QC],
                                in_=y_sb,
                            )

                def flush_norm(pend):
                    p_ot, p_l, p_h, p_j = pend
                    lsb = bsm.tile([1, QC], f32r, name="lsb")
                    nc.scalar.copy(lsb, p_l)
                    lb_ps = spsp.tile([P, 2 * QC], f32, name="sps2")[:, :QC]
                    nc.tensor.matmul(lb_ps, lhsT=col_ones, rhs=lsb, start=True, stop=True)
                    lb = bsm.tile([P, QC], f32, name="lb")
                    nc.vector.reciprocal(lb, lb_ps)
                    nc.vector.tensor_mul(OT_sb[p_h][p_j], p_ot, lb)
                    if p_h == HLOC - 1:
                        emit_oproj(p_j)

                pend = None
                for h in range(HLOC):
                    for j in range(NQC):
                        ot_ps = otpsp.tile([P, QC], f32, name="otps")
                        l_ps = lpsp.tile([1, QC], f32, name="lps")
                        nkb = 4 * (j + 1)
                        qcol0 = j * QC

                        def emit_pv(pt, kp, ot_ps=ot_ps, l_ps=l_ps, j=j, h=h, nkb=nkb):
                            for u in range(2):
                                ki = kp + u
                                diag = (ki // 4 == j)
                                cs = (ki % 4) * P if diag else 0
                                if diag:
                                    nc.gpsimd.tensor_mul(
                                        pt[:, u * QC + cs : u * QC + cs + P],
                                        pt[:, u * QC + cs : u * QC + cs + P],
                                        trimask_sb,
                                    )
                                nc.tensor.matmul(
                                    ot_ps[:, cs:],
                                    lhsT=V_sb[ki][:, h * DV : (h + 1) * DV],
                                    rhs=pt[:, u * QC + cs : (u + 1) * QC],
                                    start=(ki == 0),
                                    stop=(ki == nkb - 1),
                                )
                                nc.tensor.matmul(
                                    l_ps[:, cs:],
                                    lhsT=ones_b,
                                    rhs=pt[:, u * QC + cs : (u + 1) * QC],
                                    start=(ki == 0),
                                    stop=(ki == nkb - 1),
                                )

                        prev = None
                        for kp in range(0, nkb, 2):
                            s2 = spsp.tile([P, 2 * QC], f32, name="sps2")
                            for u in range(2):
                                ki = kp + u
                                nc.tensor.matmul(
                                    s2[:, u * QC : (u + 1) * QC],
                                    lhsT=r32(KT[h][:, ki * P : (ki + 1) * P]),
                                    rhs=r32(qTn[h][:, qcol0 : qcol0 + QC]),
                                    start=True,
                                    stop=False,
                                )
                                nc.tensor.matmul(
                                    s2[:, u * QC : (u + 1) * QC],
                                    lhsT=r32(kpe2[h][:, ki * P : (ki + 1) * P]),
                                    rhs=r32(qpeT2[:, qcol0 : qcol0 + QC]),
                                    start=False,
                                    stop=True,
                                )
                            pt = ptp.tile([P, 2 * QC], bf16, name="pt")
                            nc.scalar.activation(pt, s2, func=AF.Exp, scale=SCALE)
                            if kp == 2 and pend is not None:
                                flush_norm(pend)
                                pend = None
                            if prev is not None:
                                emit_pv(*prev)
                            prev = (pt, kp)
                        emit_pv(*prev)
                        pend = (ot_ps, l_ps, h, j)
                flush_norm(pend)
    _split_multi_waits(nc, mybir)
    return nc


def _host_prep(inputs):
    import ml_dtypes

    hs = np.ascontiguousarray(np.asarray(inputs["hidden_states"], np.float32))
    pos = np.asarray(inputs["positions"], np.int32)
    w_qa = np.asarray(inputs["w_qa"], np.float32)
    q_ln = np.asarray(inputs["q_a_ln_w"], np.float32)
    w_qb = np.asarray(inputs["w_qb"], np.float32)
    w_kva = np.asarray(inputs["w_kva"], np.float32)
    kv_ln = np.asarray(inputs["kv_a_ln_w"], np.float32)
    w_kvb = np.asarray(inputs["w_kvb"], np.float32)
    w_o = np.asarray(inputs["w_o"], np.float32)

    bf = ml_dtypes.bfloat16

    # rope tables (neox)
    inv_freq = (1.0 / (10000.0 ** (np.arange(0, DR, 2, dtype=np.float32) / DR))).astype(
        np.float32
    )
    freqs = pos.astype(np.float32)[:, None] * inv_freq[None, :]
    emb = np.concatenate([freqs, freqs], axis=-1)  # [T, 64]
    cosT = np.ascontiguousarray(np.cos(emb).T.astype(np.float32))  # [64, T]
    sinT = np.ascontiguousarray(np.sin(emb).T.astype(np.float32))
    cos2 = np.ascontiguousarray(np.concatenate([cosT, cosT], axis=0))  # [128, T]
    sin2 = np.ascontiguousarray(np.concatenate([sinT, sinT], axis=0))

    def rot_cols(A):
        return np.concatenate([-A[:, DR // 2 :], A[:, : DR // 2]], axis=1)

    # stacked stage-1 weights: kv latent | ropeA dup | ropeB dup | w_qa
    kva_lat = w_kva[:, :KVL]
    kva_rope = w_kva[:, KVL:]                       # [2048, 64]
    kva_ropeB = rot_cols(kva_rope)
    W_all = np.concatenate(
        [kva_lat, kva_rope, kva_rope, kva_ropeB, kva_ropeB, w_qa], axis=1
    )  # [2048, 512+128+128+1536]
    wS = np.ascontiguousarray(
        W_all.reshape(HID // P, P, NM1, P).transpose(1, 2, 0, 3)
    ).astype(bf)

    w_qb_f = (w_qb * q_ln[:, None]).reshape(QL, H, DQK)
    w_kvb_f = (w_kvb * kv_ln[:, None]).reshape(KVL, H, DN + DV)
    w_o_r = w_o.reshape(H, DV, HID)

    trimask = np.triu(np.ones((P, P), dtype=np.float32)).astype(bf)  # [k,q]: 1 iff q>=k

    per_core = []
    for i in range(NC_):
        hh = [HLOC * i + x for x in range(HLOC)]
        nope = np.concatenate([w_qb_f[:, h, :DN] for h in hh], axis=1)  # [QL,256]
        ropeA = np.concatenate([w_qb_f[:, h, DN:] for h in hh], axis=1)  # [QL,128]
        ropeB = np.concatenate(
            [rot_cols(w_qb_f[:, h, DN:]) for h in hh], axis=1
        )
        wqb_aug = np.ascontiguousarray(
            np.concatenate([nope, ropeA, ropeB], axis=1)
            .reshape(QL // P, P, 4, P)
            .transpose(1, 2, 0, 3)
        ).astype(bf)  # [p, mo, k, j]
        wkvbk = np.ascontiguousarray(
            np.concatenate([w_kvb_f[:, h, :DN] for h in hh], axis=1)
            .reshape(KVL // P, P, HLOC * DN)
            .transpose(1, 0, 2)
        ).astype(bf)
        wkvbv = np.ascontiguousarray(
            np.concatenate([w_kvb_f[:, h, DN:] for h in hh], axis=1)
            .reshape(KVL // P, P, HLOC * DV)
            .transpose(1, 0, 2)
        ).astype(bf)
        wo_i = np.ascontiguousarray(
            np.stack([w_o_r[h] for h in hh], axis=0).transpose(1, 0, 2)
        )  # [p, h, HID]
        hL = np.ascontiguousarray(
            hs[i * TLOC : (i + 1) * TLOC].reshape(TLOC, HID // P, P).transpose(2, 1, 0)
        ).astype(bf)  # [p, k, t]
        per_core.append(
            dict(
                hL=hL,
                wS=wS,
                cosL=np.ascontiguousarray(cos2[:, i * TLOC : (i + 1) * TLOC]),
                sinL=np.ascontiguousarray(sin2[:, i * TLOC : (i + 1) * TLOC]),
                wqb=wqb_aug,
                wkvbk=wkvbk,
                wkvbv=wkvbv,
                wo=wo_i,
                cos2=cos2,
                sin2=sin2,
                trimask=trimask,
            )
        )
    return per_core


def kernel(**inputs):
    global LAST_RESULTS
    from concourse.bass_utils import run_bass_kernel_spmd

    if "nc" not in _CACHE:
        _CACHE["nc"] = _build_program()
    nc = _CACHE["nc"]

    in_maps = _host_prep(inputs)
    res = run_bass_kernel_spmd(nc, in_maps, core_ids=list(range(NC_)))
    LAST_RESULTS = res
    out = np.zeros((T, HID), dtype=np.float32)
    for r in res.results:
        out += np.asarray(r["y"], dtype=np.float32)
    return out
